# revision 9
# baseline (speedup 1.0000x reference)
"""Trainium2 Bass kernel for BSplineActivation (KAN-style activation).

Reference computation (G=3 grid points on [-1,1], NUM_CP=5, degree 4):
    t        = clip(x, -1, 1)
    y_spline = lerp of s[floor], s[ceil]   where s[g] = basis_values[g] @ control_points
    out      = base_weight * silu(x) + spline_weight * y_spline

Because G=3, y_spline is piecewise LINEAR in t with breakpoints {-1, 0, 1}:
with A = sw*(s2-s1), B = sw*(s1-s0), c = sw*s1 (host-computed scalars):
    out = bw*silu(x) + c + L(t),   L(t) = A*t (t>=0) else B*t

The whole pipeline runs in fp16 (the harness gate is rel_err < 2e-2; fp16
end-to-end lands ~4e-4): x is downcast on host, the device reads/writes
fp16 HBM buffers, and the host upcasts the result. That halves HBM traffic
vs fp32 (8.4 MiB in + 8.4 MiB out per core, ~55 us/sweep at the measured
~305 GB/s/core sustained).

Per-core device program (pure data parallel, 1 batch/core), two tile paths:
  A-path (Prelu on ACT):              B-path (spline on DVE, first tile):
    u = Silu(x)            ACT          u = Silu(x)            ACT
    t = TS(x, max-1, min1) DVE 4x       p = TS(x,*A,max).TS(min)  DVE
    w = Prelu(sw*t, alpha) ACT          n = TS(x,*B,max).TS(min)  DVE
    u = u*bw + c           DVE          u = u*bw + c              DVE
    o = u (+|-) w          DVE 2x       o = (p TT+ n) TT+ u       DVE
ACT is the bottleneck engine (~32.7 us per pass, dtype-independent, vs
~55-56 us DMA and ~34 us DVE for the A-chain); routing tiles 0 and 3
through the B-path cuts ACT to 1.75 passes (~57 us busy). B-tile chains
are emitted per column-half (BCOLSPLIT=2) - full-tile B-chains are ~12 us
of serial DVE latency and caused pipeline stalls when more than one B-tile
was in flight, and their in-DMAs land per column-half too (BDMASPLIT) so
the first Silu starts ~1.5 us earlier. Measured sustained sweep (K=1025
slope): ~62-64 us, vs 65.7 without the DMA split, 69.1 for all-A fp16 and
98.55 us for the fp32 baseline. Pure fp16 DMA copy floor on this tiling:
~55-56 us/sweep. Measured dead ends: 3-4 B-tiles, quarter-split B chains,
splitting every in-DMA, colsplit8 tail, B-path tail tile - all regressed.
B-tile PLACEMENT matters via tile-pool rotation: with BUFS_MID=2, tile i
uses pmid buffer i%2, and two B-tiles on the same parity (e.g. 0,2 or 3,5)
serialize on the long-held B-chain buffers (+2-7 us); 0,3 avoids it. A
third B-tile at 6 (parity-safe) measures neutral: ACT drops to 1.625
passes but DMA (~55.5) then binds and the sweep stays ~62 us. Moving the
tail tiles (6-8) to a separate tile pool to decouple the next iteration's
first in-DMA from this iteration's tail also regressed (+1.5 us).
Tapered tail (rowsplit, then geometrically column-split final tile
1024/512/256/256) shortens the pipeline drain. The K-repeat loop uses
For_i(staggered_reset=True): no all-engine barrier on the back edge, so
consecutive sweeps overlap (-1.5 us/sweep, outputs verified bit-identical
at K=3 and K=1025).
"""

import numpy as np

# Problem shape (hardcoded; kernel.py must be self-contained).
BATCH = 8
ROWS = 2048
COLS = 2048
P = 128  # SBUF partitions
ROW_BLOCKS = ROWS // P  # 16 row-blocks of [128, 2048] per core
# Per-tile (row_blocks, mode, path). Tapered tail shortens pipeline drain:
#  - 'whole':    one TT + one out-DMA for the whole tile
#  - 'rowsplit': TT + out-DMA per row-block (out starts after half the TT)
#  - 'colsplit4': full chain per column-quarter (shortest drain, last tile)
# path 'B' computes the spline on DVE (no Prelu) - used for the first tile
# so ACT (the bottleneck engine) starts a pass-eighth shorter.
SCHEDULE = ([(2, "whole", "B")] + [(2, "whole", "A")] * 2 +
            [(2, "whole", "B")] + [(2, "whole", "A")] * 2 +
            [(2, "rowsplit", "A"), (1, "whole", "A"), (1, "colsplitG", "A")])
BCOLSPLIT = 2   # B-tile chains emitted per column-half
BDMASPLIT = True
STAGGERED = True  # split B-tile in-DMAs per half: ACT starts ~1.5 us earlier
BUFS_IO = 3   # x-in / out tiles: triple buffering
BUFS_MID = 2  # intermediate tiles


def _build_nc(bw, c, scale_w, alpha, sign, A, B, repeat=1):
    import concourse.bacc as bacc
    import concourse.mybir as mybir
    from concourse.tile import TileContext

    f16 = mybir.dt.float16
    AF = mybir.ActivationFunctionType
    ALU = mybir.AluOpType

    assert sum(nb for nb, _, _ in SCHEDULE) == ROW_BLOCKS

    nc = bacc.Bacc("TRN2")
    x = nc.dram_tensor("x", [ROWS, COLS], f16, kind="ExternalInput")
    out = nc.dram_tensor("out", [ROWS, COLS], f16, kind="ExternalOutput")
    xv = x.rearrange("(a p) f -> a p f", p=P)     # [16, 128, 2048]
    ov = out.rearrange("(a p) f -> a p f", p=P)
    tt_op = ALU.add if sign > 0 else ALU.subtract

    def compute_a(xt, u, t, w, o):
        """A-path elementwise chain on matching AP slices."""
        nc.scalar.activation(out=u, in_=xt, func=AF.Silu)
        nc.vector.tensor_scalar(out=t, in0=xt, scalar1=-1.0, scalar2=1.0,
                                op0=ALU.max, op1=ALU.min)
        nc.scalar.activation(out=w, in_=t, func=AF.Prelu,
                             scale=float(scale_w), alpha=float(alpha))
        nc.vector.tensor_scalar(out=u, in0=u, scalar1=float(bw),
                                scalar2=float(c), op0=ALU.mult, op1=ALU.add)
        nc.vector.tensor_tensor(out=o, in0=u, in1=w, op=tt_op)

    def compute_b(xt, u, t, w, o):
        """B-path: out = bw*u + c + A*clip(x,0,1) + B*clip(x,-1,0), all on
        DVE. s*clip(x,lo,hi) == clip(s*x, ...) with sign-sorted bounds."""
        nc.scalar.activation(out=u, in_=xt, func=AF.Silu)
        nc.vector.tensor_scalar(out=t, in0=xt, scalar1=float(A),
                                scalar2=float(min(0.0, A)),
                                op0=ALU.mult, op1=ALU.max)
        nc.vector.tensor_scalar(out=t, in0=t, scalar1=float(max(0.0, A)),
                                scalar2=None, op0=ALU.min)
        nc.vector.tensor_scalar(out=w, in0=xt, scalar1=float(B),
                                scalar2=float(min(-B, 0.0)),
                                op0=ALU.mult, op1=ALU.max)
        nc.vector.tensor_scalar(out=w, in0=w, scalar1=float(max(-B, 0.0)),
                                scalar2=None, op0=ALU.min)
        nc.vector.tensor_scalar(out=u, in0=u, scalar1=float(bw),
                                scalar2=float(c), op0=ALU.mult, op1=ALU.add)
        nc.vector.tensor_tensor(out=t, in0=t, in1=w, op=ALU.add)
        nc.vector.tensor_tensor(out=o, in0=t, in1=u, op=ALU.add)

    def body(pio, pmid):
        s = 0
        for nb, mode, path in SCHEDULE:
            compute = compute_b if path == "B" else compute_a
            shape = [P, nb, COLS] if nb > 1 else [P, COLS]
            src = (xv[s] if nb == 1
                   else xv[s:s + nb].rearrange("b p f -> p b f"))

            xt = pio.tile(shape, f16, tag="xt")
            split_in = (BDMASPLIT and nb > 1 and mode == "whole" and
                        path == "B")
            if split_in:
                H = COLS // BCOLSPLIT
                for h in range(BCOLSPLIT):
                    csl = (slice(None), slice(None), slice(h * H, (h + 1) * H))
                    nc.sync.dma_start(out=xt[csl], in_=src[csl])
            else:
                nc.sync.dma_start(out=xt, in_=src)
            u = pmid.tile(shape, f16, tag="u")
            t = pmid.tile(shape, f16, tag="t")
            w = pmid.tile(shape, f16, tag="w")
            o = pio.tile(shape, f16, tag="o")

            if mode.startswith("colsplit"):
                assert nb == 1
                if mode == "colsplitG":
                    widths = [1024, 512, 256, 256]
                else:
                    ncs = int(mode[len("colsplit"):])
                    widths = [COLS // ncs] * ncs
                off = 0
                for wd in widths:
                    sl = slice(off, off + wd)
                    compute(xt[:, sl], u[:, sl], t[:, sl], w[:, sl], o[:, sl])
                    nc.sync.dma_start(out=ov[s][:, sl], in_=o[:, sl])
                    off += wd
            elif mode == "rowsplit" and nb > 1:
                assert path == "A"
                nc.scalar.activation(out=u, in_=xt, func=AF.Silu)
                nc.vector.tensor_scalar(out=t, in0=xt, scalar1=-1.0,
                                        scalar2=1.0, op0=ALU.max, op1=ALU.min)
                nc.scalar.activation(out=w, in_=t, func=AF.Prelu,
                                     scale=float(scale_w), alpha=float(alpha))
                nc.vector.tensor_scalar(out=u, in0=u, scalar1=float(bw),
                                        scalar2=float(c), op0=ALU.mult,
                                        op1=ALU.add)
                for b in range(nb):
                    nc.vector.tensor_tensor(out=o[:, b, :], in0=u[:, b, :],
                                            in1=w[:, b, :], op=tt_op)
                    nc.sync.dma_start(out=ov[s + b], in_=o[:, b, :])
            elif path == "B" and mode == "whole" and BCOLSPLIT > 1:
                H = COLS // BCOLSPLIT
                ovt = (ov[s] if nb == 1
                       else ov[s:s + nb].rearrange("b p f -> p b f"))
                for h in range(BCOLSPLIT):
                    sl = (slice(None), slice(h * H, (h + 1) * H)) if nb == 1 \
                        else (slice(None), slice(None), slice(h * H, (h + 1) * H))
                    compute(xt[sl], u[sl], t[sl], w[sl], o[sl])
                    nc.sync.dma_start(out=ovt[sl], in_=o[sl])
            else:
                dst = (ov[s] if nb == 1
                       else ov[s:s + nb].rearrange("b p f -> p b f"))
                compute(xt, u, t, w, o)
                nc.sync.dma_start(out=dst, in_=o)
            s += nb

    with TileContext(nc) as tc:
        with tc.tile_pool(name="pio", bufs=BUFS_IO) as pio, \
             tc.tile_pool(name="pmid", bufs=BUFS_MID) as pmid:
            if repeat == 1:
                body(pio, pmid)
            else:
                with tc.For_i(0, repeat, 1, staggered_reset=STAGGERED):
                    body(pio, pmid)

    nc.compile()
    return nc


def _host_constants(control_points, base_weight, spline_weight, basis_values):
    cp = np.asarray(control_points, dtype=np.float64)
    bv = np.asarray(basis_values, dtype=np.float64)
    bw = float(np.asarray(base_weight).reshape(-1)[0])
    sw = float(np.asarray(spline_weight).reshape(-1)[0])
    s = bv @ cp  # s[g] = dot(basis_values[g], control_points), g in {0,1,2}
    c = sw * s[1]
    A = sw * (s[2] - s[1])  # slope for t >= 0
    B = sw * (s[1] - s[0])  # slope for t < 0
    if A != 0.0:
        scale_w, alpha, sign = abs(A), B / A, (1.0 if A > 0 else -1.0)
    elif B != 0.0:
        scale_w, alpha, sign = (-B if B > 0 else B), 0.0, (-1.0 if B > 0 else 1.0)
    else:
        scale_w, alpha, sign = 0.0, 0.0, 1.0
    return bw, c, scale_w, alpha, sign, A, B


def _kernel_nc_and_inputs(x, control_points, base_weight, spline_weight,
                          basis_values, _repeat=1):
    x = np.asarray(x)
    assert x.shape == (BATCH, ROWS, COLS), x.shape
    x16 = np.ascontiguousarray(x.astype(np.float16))
    bw, c, scale_w, alpha, sign, A, B = _host_constants(
        control_points, base_weight, spline_weight, basis_values
    )
    nc = _build_nc(bw, c, scale_w, alpha, sign, A, B, repeat=_repeat)
    in_maps = [{"x": x16[i]} for i in range(BATCH)]
    return nc, in_maps


def kernel(x, control_points, base_weight, spline_weight, basis_values,
           _repeat=1, _return_nc=False):
    from concourse.bass_utils import run_bass_kernel_spmd

    nc, in_maps = _kernel_nc_and_inputs(
        x, control_points, base_weight, spline_weight, basis_values,
        _repeat=_repeat,
    )
    if _return_nc:
        return nc
    res = run_bass_kernel_spmd(nc, in_maps, core_ids=list(range(BATCH)))
    out = np.stack([res.results[i]["out"] for i in range(BATCH)], axis=0)
    return out.astype(np.float32)
